# revision 34
# baseline (speedup 1.0000x reference)
"""Trainium2 Bass kernel for the ragged-sequence cross-attention module.

Math (reference):
    f       = Wf @ f_pre_in.T + bf                      (H, M)
    b_feat  = Wb @ b_pre_in[g] + bb                     per graph (H, N)
    bv_feat = Wbv @ bv_in[g] + bbv                      per graph (H, N)
    w_euc   = softmax((b_feat[g,:32].T @ f[:32]) / 8)   per node over N
    w_geo   = softmax((b_feat[g,32:].T @ f[32:]) / 8)
    out     = [bv_feat[g] @ w_euc, bv_feat[g] @ w_geo] @ Wo.T + bo   (M, H)

Algebraic folds used here (host-side weight preprocessing only):
  * bb never enters: softmax over n is invariant to per-node constants, so
    s = (b^ + bb)^T (f^ + bf) == b^T (f^ + bf) up to a per-node constant.
  * Wbv/bbv/Wo/bo fold into A = Wo[:, :H] @ Wbv, Bm = Wo[:, H:] @ Wbv and
    bo_tot = bo + (Wo[:, :H] + Wo[:, H:]) @ bbv, since sum(softmax) == 1.
    The device then applies raw bv (6 channels) + a ones channel
    (denominator) and projects [e/de; 1; g/dg; 1] with a (14 -> 64) matmul.

Sharding: one core per graph (B == n_cores == 8).  Nodes are sorted by
graph, so core g takes graph g's contiguous node range, padded to
T = max_g ceil(len_g/128) m-tiles of 128 nodes.  Each core loads only its
own graph's b_pre/bv plus its node features: ~0.3 MB (bf16/fp8 staging).

Device pipeline per m-tile t (128 nodes, n = 512 boundary positions):
  scores  s[n, m] = b^T f^  -- 8 bf16 matmuls into PSUM (128, 2, 4, 128)
  exp     one ACT op (128, 1024): te = exp(s/8) -> fp8e4, DoubleRow layout
  apply   fp8 DoubleRow matmuls: lhsT = [bv(6) | ones x7 | 0 x3] (128,2,16)
          -> psum rows 0..5 = raw outs, rows 6..12 = denominator (replicated
          on 7 partitions so no partition-broadcast is ever needed)
  norm    DVE: reciprocal (7, ...) then two tensor_muls -> cat (14, t, 128)
          rows 0..5 = e/de, 6 = 1 (de/de), 7..12 = g/dg, 13 = 1
  fin     one (14 -> 64) bf16 matmul (bo_tot in row 6) -> PSUM -> DMA out

Norm/fin run in batches ([0:T-2], [T-2:T-1], [T-1:T]) so the serial tail
after the last exp is one small recip+mul+matmul+DMA chain.
"""

import sys

for _p in ("/opt/trn_rl_repo", "/root/.axon_site/_ro/trn_rl_repo"):
    if _p not in sys.path:
        sys.path.append(_p)

import numpy as np

import bass_rust

import concourse.bass as bass
import concourse.mybir as mybir
from concourse.bass_utils import run_bass_kernel_spmd
from concourse.tile import TileContext
from concourse.vector_clock import ScopedClock, VectorClock

F32 = mybir.dt.float32
BF16 = mybir.dt.bfloat16
FP8 = mybir.dt.float8e4

NP_BF16 = mybir.dt.np(BF16)
NP_FP8 = mybir.dt.np(FP8)

# Problem shapes (hardcoded per the harness contract).
M, B, N, FD, BD, BVD, H = 4096, 8, 512, 128, 128, 6, 64
H2 = H // 2
N_CORES = 8

# Apply matmuls run fp8 DoubleRow (2 k-tiles per pass).  Set False to fall
# back to plain fp8 single-row accumulation (4 matmuls per half).
USE_DR = True

# Tile indices whose exp runs on DVE as a Schraudolph affine map straight to
# fp8e4 bits (byte = round(s*log2e + 56.+c)), offloading the bottleneck ACT
# engine.  The mantissa-linear approximation costs ~3% extra weight noise on
# those tiles only.  Set to () to keep every exp on ACT.
EXP_DVE_TILES = ()
# fp8e4(v) bits = 8*(log2(v)+7) with mantissa-linear interpolation; the +0.36
# centers the 2^frac-1 vs frac error band.
SCHRAUDOLPH_MUL = 1.4426950408889634  # log2(e): byte per unit score (s/8*8)
SCHRAUDOLPH_ADD = 56.5 + 0.36  # +0.5 assumes truncating f32->u8 convert

# The walrus build in this environment rejects multiple semaphore waits
# on one instruction, so carry every wait on its own nop ahead of the real
# instruction.
_MAX_WAITS = 1


class _ChunkedDrainTileContext(TileContext):
    """The walrus build in this environment rejects >1 semaphore wait on a
    single instruction, which breaks TileContext's final drain (it carries
    one wait per touched proc).  Split those waits across one nop per proc;
    each engine executes serially, so a bare drain afterwards is
    equivalent."""

    _nop_uid = 0

    def _add_instruction(self, inst):
        si = inst.sync_info
        if (
            si is not None
            and si.on_wait
            and len(si.on_wait) > _MAX_WAITS
            and inst.engine != mybir.EngineType.Unassigned
        ):
            waits = list(si.on_wait)
            excess, keep = waits[:-_MAX_WAITS], waits[-_MAX_WAITS:]
            for i in range(0, len(excess), _MAX_WAITS):
                _ChunkedDrainTileContext._nop_uid += 1
                nop = mybir.InstNoOp(
                    name=f"splitw{_ChunkedDrainTileContext._nop_uid}", ins=[], outs=[]
                )
                nop.engine = inst.engine
                nop.sync_info = bass_rust.SyncInfo(
                    on_wait=excess[i : i + _MAX_WAITS], on_update=[]
                )
                super()._add_instruction(nop)
            inst.sync_info = bass_rust.SyncInfo(on_wait=keep, on_update=si.on_update)
        super()._add_instruction(inst)

    def _drain_and_barrier(self, tick_clock, wait_clock):
        nc = self.nc
        g = tick_clock.global_clock
        nprocs = len(g)
        for i in range(nprocs):
            if g[i] > 0:
                vc = VectorClock([0] * nprocs)
                vc.require_at_least(i, g[i])
                nop_inst = nc.sync.nop(nofuse=True, hint=f"drain_wait_p{i}")
                wait_clock.add_sem_waits(nop_inst.ins, ScopedClock({None: vc}))
        nc.sync.drain()
        nc.all_engine_barrier()
        assert self.sems is not None
        popped = nc._tile_sem_poison_stack.pop()
        assert popped is self._sem_poison
        nc.clear_and_free_semaphores(list(self.sems.allocated().values()))
        nc.all_engine_barrier()


def _tile_widths(maxL):
    """Column widths of the per-core m-tiles: all 128 wide.  Sub-128
    matmul widths abort on this hardware (PSUM write constraint the sim
    does not model), so the remainder tile stays 128 wide for every PE op
    and only its exp activation is sliced down (see build_program)."""
    return [128] * max(1, -(-maxL // 128))


def _batches(nt):
    """Group tile indices into norm/fin batches of <=2 tiles.  The last
    batch naturally ends up being the (small) remainder tile, keeping the
    post-last-exp serial tail short."""
    return [(i, min(i + 2, nt)) for i in range(0, nt, 2)]


def build_program(maxL, reps=1):
    """Build the per-core SPMD Bass program (identical on all 8 cores).

    maxL = max nodes per graph (sets the padded slot count).  reps>1
    repeats the pipeline writing to distinct output slices (timing only)."""
    nc = bass.Bass()
    widths = _tile_widths(maxL)
    offs = [0]
    for w in widths:
        offs.append(offs[-1] + w)
    TM = offs[-1]
    NT = len(widths)
    # The last tile is mostly padding (maxL mod 128 real nodes); its exp --
    # the expensive ACT op -- only covers the real columns, rounded up a
    # little.  Stale te bytes beyond that feed only discarded padding slots.
    wr = maxL - 128 * (NT - 1)
    exp_w = [128] * NT
    if 0 < wr < 128:
        exp_w[NT - 1] = min(128, -(-wr // 16) * 16)

    d_w = nc.declare_dram_parameter("wpk", [128, 256], BF16, isOutput=False)
    d_bf = nc.declare_dram_parameter("bfv", [H, 1], F32, isOutput=False)
    d_ft = nc.declare_dram_parameter("ft", [FD, TM], BF16, isOutput=False)
    d_bp = nc.declare_dram_parameter("bp", [BD, N], BF16, isOutput=False)
    d_bv = nc.declare_dram_parameter("bv8", [128, 4, 64], FP8, isOutput=False)
    d_out = nc.declare_dram_parameter("outT", [reps, H, TM], BF16, isOutput=True)

    DR = mybir.MatmulPerfMode.DoubleRow if USE_DR else None
    Exp = mybir.ActivationFunctionType.Exp

    with _ChunkedDrainTileContext(nc) as tc, nc.allow_low_precision(
        reason="bf16/fp8 rounding of fp32 data"
    ):
        with (
            tc.tile_pool(name="const", bufs=1) as cp,
            tc.tile_pool(name="work", bufs=2) as wkp,
            tc.tile_pool(name="te", bufs=3) as tep,
            tc.tile_pool(name="ps_s", bufs=2, space="PSUM") as pss,
            tc.tile_pool(name="ps_a", bufs=2, space="PSUM") as psap,
        ):
            # Preload the ACT Exp table at t=0 (a real first Exp would pay
            # the ~1.3us table load on the critical path).
            t_warm = cp.tile([1, 1], F32, tag="warm")
            nc.vector.memset(t_warm[:], 0.0)
            t_warm2 = cp.tile([1, 1], F32, tag="warm2")
            nc.scalar.activation(t_warm2[:], t_warm[:], Exp)

            # b_pre first on the SWDGE queue (bhat gates the first scores),
            # as two tiles so the half-0 matmul starts sooner.
            t_bpA = cp.tile([BD, N // 2], BF16, tag="bpA")
            nc.gpsimd.dma_start(t_bpA[:], d_bp[:, 0 : N // 2])
            t_bpB = cp.tile([BD, N // 2], BF16, tag="bpB")
            nc.gpsimd.dma_start(t_bpB[:], d_bp[:, N // 2 : N])
            t_bv = cp.tile([128, 4, 64], FP8, tag="bv8")
            nc.gpsimd.dma_start(t_bv[:], d_bv[:])
            t_bf = cp.tile([H, 1], F32, tag="bfv")
            nc.gpsimd.dma_start(t_bf[:], d_bf[:])
            t_w = cp.tile([128, 256], BF16, tag="wpk")
            nc.scalar.dma_start(t_w[:], d_w[:])

            t_wft = t_w[:, 0:64]
            t_wbt = t_w[:, 64:128]
            # final projection lhsT: rows 0-6 = [A.T; bo_tot], 32-38 = [Bm.T; 0]
            t_wfe = t_w[0:7, 128:192]
            t_wfg = t_w[0:7, 192:256]

            # fhat pieces: a small first piece so tile-0 scores start early.
            pieces = [(0, min(128, TM))]
            if TM > 128:
                pieces.append((128, TM))

            for rep in range(reps):
                # ---- per-rep node-feature load (SP queue, piecewise) ----
                t_ft = wkp.tile([FD, TM], BF16, tag="ft")
                for s, e in pieces:
                    nc.sync.dma_start(t_ft[:, s:e], d_ft[:, s:e])

                # ---- features ----
                # bhat first on PE (it gates the first scores); two separate
                # half-tiles so the first scores only wait on half 0's copy,
                # which itself overlaps the half-1 matmul
                t_bh0 = wkp.tile([H, N // 2], BF16, tag="bh0")
                t_bh1 = wkp.tile([H, N // 2], BF16, tag="bh1")
                ps_b0 = psap.tile([H, N // 2], F32, tag="finA", bufs=1)
                ps_b1 = psap.tile([H, N // 2], F32, tag="a")
                def bhat(h0, j):  # noqa: B023 - per-rep closure
                    t_bh = t_bh0 if j < 2 else t_bh1
                    return t_bh[h0 : h0 + H2, 128 * (j % 2) : 128 * (j % 2 + 1)]

                # fhat in two tiles so early scores don't wait on the rest
                t_fh0 = wkp.tile([H, pieces[0][1]], BF16, tag="fh0")
                if TM > 128:
                    t_fh1 = wkp.tile([H, TM - 128], BF16, tag="fh1")
                else:
                    t_fh1 = None

                def fhat(h0, s, e):  # noqa: B023 - per-rep closure
                    if e <= 128:
                        return t_fh0[h0 : h0 + H2, s:e]
                    assert s >= 128
                    return t_fh1[h0 : h0 + H2, s - 128 : e - 128]

                # PE order: mm_bA, ps_f piece0, mm_bB, ps_f rest -- ft piece
                # 0 lands before b_pre half B does, so slotting its matmul
                # between the two bhat matmuls starts fadd0 (which gates ALL
                # scores) ~200ns earlier at no cost to the bhat chain.
                nc.tensor.matmul(
                    ps_b0[:], t_wbt, t_bpA[:], start=True, stop=True
                )
                # ACT is idle before the exps; keep DVE off this path
                nc.scalar.copy(t_bh0[:], ps_b0[:])
                (s0, e0) = pieces[0]
                ps_f0 = psap.tile([H, e0 - s0], F32, tag="a", bufs=2)
                nc.tensor.matmul(
                    ps_f0[:], t_wft, t_ft[:, s0:e0], start=True, stop=True
                )
                nc.vector.tensor_scalar_add(t_fh0[:], ps_f0[:], t_bf[:])
                nc.tensor.matmul(
                    ps_b1[:], t_wbt, t_bpB[:], start=True, stop=True
                )
                nc.scalar.copy(t_bh1[:], ps_b1[:])
                for (s, e), t_fh in zip(pieces[1:], (t_fh1,)):
                    ps_f = psap.tile([H, e - s], F32, tag="finA", bufs=1)
                    nc.tensor.matmul(
                        ps_f[:], t_wft, t_ft[:, s:e], start=True, stop=True
                    )
                    nc.vector.tensor_scalar_add(t_fh[:], ps_f[:], t_bf[:])

                # Natural order: every full exp (~1038ns) covers the next
                # tile's scores latency, so the ps_s ring of 2 never gaps,
                # and the (cheap, sliced) remainder tile lands LAST so the
                # post-last-exp serial tail is the short sliced chain.
                tile_order = list(range(NT))
                last_t = tile_order[-1]

                # Final projections land in two PSUM pieces: finA holds every
                # tile except the last-processed one (packed 128-col slots),
                # so its SBUF copies + out-DMAs can leave while the last
                # tile's norm still runs; finB (one tile) follows.  For
                # NT > 5 the split would overflow the 8 PSUM banks, so fall
                # back to one fin tile and a single tail copy/DMA.
                split_fin = 2 <= NT <= 5
                if split_fin:
                    nonlast = [t for t in tile_order if t != last_t]
                    ps_finB = psap.tile([H, 128], F32, tag="finB", bufs=1)
                else:
                    nonlast = list(tile_order)
                    ps_finB = None
                fcol = {t: 128 * i for i, t in enumerate(sorted(nonlast))}
                ps_finA = psap.tile(
                    [H, 128 * len(nonlast)], F32, tag="finA", bufs=1
                )

                # psa rows 0-5 = raw outs, 6 = denom, 32-38 = denom x7
                # (replicated via ones columns in bv8 so the reciprocal and
                # the tensor_muls each see quadrant-aligned partition bases)
                norm_tiles = {}

                def apply_norm_tile(t, te, hi=None, defer_fin=False):  # noqa: B023
                    # hi=None: both halves.  hi=0/1: just that half's apply
                    # matmuls + sliced recip/mul (the fins land at hi=1), so
                    # the long-tail tile's h0 norm overlaps its h1 exp.
                    w = widths[t]
                    if t not in norm_tiles:
                        norm_tiles[t] = (
                            psap.tile([64, 2, w], F32, tag="a", name=f"psa{t}"),
                            wkp.tile([64, 2, w], F32, tag="rcp", name=f"rcp{t}"),
                            wkp.tile([64, 2, w], BF16, tag="cat", name=f"cat{t}"),
                        )
                    t_psa, t_rcp, t_cat = norm_tiles[t]
                    his = range(2) if hi is None else [hi]
                    for h in his:
                        if USE_DR:
                            for jp in range(2):
                                nc.tensor.matmul(
                                    t_psa[0:64, h, :],
                                    t_bv[:, 2 * jp : 2 * jp + 2, :],
                                    te[:, h, 2 * jp : 2 * jp + 2, :],
                                    start=(jp == 0),
                                    stop=(jp == 1),
                                    perf_mode=DR,
                                )
                        else:
                            for j in range(4):
                                nc.tensor.matmul(
                                    t_psa[0:64, h, :],
                                    t_bv[:, j, :],
                                    te[:, h, j, :],
                                    start=(j == 0),
                                    stop=(j == 3),
                                )
                    # norm: cat (7, 2, w) rows 0-5 = x/dx, row 6 = 1, with the
                    # half index in the free dim.  The DVE ops only cover the
                    # real columns of the remainder tile; stale cat columns
                    # beyond feed discarded padding slots.
                    ew = exp_w[t]
                    hs = slice(None) if hi is None else slice(hi, hi + 1)
                    nc.vector.reciprocal(
                        t_rcp[32:39, hs, 0:ew], t_psa[32:39, hs, 0:ew]
                    )
                    if hi is None and ew < w:
                        nc.gpsimd.memset(t_cat[:, :, ew:w], 0.0)
                    nc.vector.tensor_mul(
                        t_cat[0:7, hs, 0:ew],
                        t_psa[0:7, hs, 0:ew],
                        t_rcp[32:39, hs, 0:ew],
                    )
                    if hi == 0:
                        return
                    if not defer_fin:
                        emit_fin(t)

                def emit_fin(t):  # noqa: B023
                    # Separate from apply_norm_tile: the fin matmul waits on
                    # the DVE mul, so on the in-order PE queue it must sit
                    # AFTER the later tiles' apply matmuls or it stalls them.
                    w = widths[t]
                    t_cat = norm_tiles[t][2]
                    if split_fin and t == last_t:
                        fin = ps_finB[:, 0:w]
                    else:
                        fin = ps_finA[:, fcol[t] : fcol[t] + w]
                    nc.tensor.matmul(
                        fin, t_wfe, t_cat[0:7, 0, :], start=True, stop=False
                    )
                    nc.tensor.matmul(
                        fin, t_wfg, t_cat[0:7, 1, :], start=False, stop=True
                    )

                # scores -> exp -> (apply+norm) with one-tile lookahead:
                # engines run their queues in order, so apply(t-1) is emitted
                # after scores(t) to keep PE busy during exp(t-1).
                # The long-tail tile (the last FULL one when the remainder
                # is last) runs its exp as two half ops so its h0 apply/norm
                # overlaps the h1 exp and the remainder tile's chain.
                tl = None  # half-split of the long-tail tile regressed:
                # ps_finA dependencies are whole-tile, so every out copy
                # waits for the last fin regardless; keep the simple path.
                pend = None  # (t, te) whose apply/norm is not yet emitted
                te_tl = None
                for t in tile_order:
                    w = widths[t]
                    ps_s = pss.tile([128, 2, 4, w], F32, tag="s")
                    for hi in range(2):
                        h0 = H2 * hi
                        for j in range(4):
                            nc.tensor.matmul(
                                ps_s[:, hi, j, :],
                                bhat(h0, j),
                                fhat(h0, offs[t], offs[t] + w),
                                start=True,
                                stop=True,
                            )
                    te = tep.tile([128, 2, 4, w], FP8, tag="te")
                    ew = exp_w[t]
                    if ew < w:
                        # padding columns the sliced exp skips: zero once on
                        # the idle Pool engine (their outputs are discarded,
                        # but stale bytes must not be NaN for the sim)
                        nc.gpsimd.memset(te[:, :, :, ew:w], 0.0)
                    if t == tl:
                        nc.scalar.activation(
                            te[:, 0:1, :, :], ps_s[:, 0:1, :, :], Exp, scale=0.125
                        )
                        if pend is not None:
                            apply_norm_tile(*pend)
                        nc.scalar.activation(
                            te[:, 1:2, :, :], ps_s[:, 1:2, :, :], Exp, scale=0.125
                        )
                        pend = None
                        te_tl = te
                        continue
                    if t in EXP_DVE_TILES:
                        nc.vector.tensor_scalar(
                            te[:, :, :, 0:ew].bitcast(mybir.dt.uint8),
                            ps_s[:, :, :, 0:ew],
                            SCHRAUDOLPH_MUL,
                            SCHRAUDOLPH_ADD,
                            mybir.AluOpType.mult,
                            mybir.AluOpType.add,
                        )
                    else:
                        nc.scalar.activation(
                            te[:, :, :, 0:ew], ps_s[:, :, :, 0:ew], Exp, scale=0.125
                        )
                    if pend is not None:
                        pt, pte = pend
                        apply_norm_tile(pt, pte)
                        if pt >= NT - 1 is False and pt >= NT - 2:
                            pass
                    if te_tl is not None:
                        apply_norm_tile(tl, te_tl, hi=0)
                        apply_norm_tile(tl, te_tl, hi=1)
                        te_tl = None
                    pend = (t, te)
                if te_tl is not None:
                    apply_norm_tile(tl, te_tl, hi=0)
                    apply_norm_tile(tl, te_tl, hi=1)
                if pend is not None:
                    pt, pte = pend
                    apply_norm_tile(pt, pte)


                # Piece-wise copies + SWDGE out-DMAs: finA's contiguous runs
                # leave as soon as their norms finish; finB (the last tile's
                # norm chain) goes last.
                t_fout = wkp.tile([H, TM], BF16, tag="fout")
                # finA leaves in two pieces so the big early piece's copy +
                # DMA never wait on the long-tail tile's norm chain.
                runs = []  # contiguous tile runs of finA
                for t in sorted(nonlast):
                    if runs and runs[-1][1] == t:
                        runs[-1][1] = t + 1
                    else:
                        runs.append([t, t + 1])
                pieces = []
                for t0, t1 in runs:
                    if t1 - t0 > 1 and t1 - 1 == tl:
                        pieces.extend([(t0, t1 - 1), (t1 - 1, t1)])
                    else:
                        pieces.append((t0, t1))
                for t0, t1 in pieces:
                    nc.scalar.copy(
                        t_fout[:, offs[t0] : offs[t1]],
                        ps_finA[:, fcol[t0] : fcol[t0] + offs[t1] - offs[t0]],
                    )
                    nc.sync.dma_start(
                        d_out[rep][:, offs[t0] : offs[t1]],
                        t_fout[:, offs[t0] : offs[t1]],
                    )
                if ps_finB is not None:
                    # DVE frees up exactly when the last norm finishes; the
                    # remainder's piece is sliced to its real columns.
                    e = exp_w[last_t]
                    nc.vector.tensor_copy(
                        t_fout[:, offs[last_t] : offs[last_t] + e],
                        ps_finB[:, 0:e],
                    )
                    nc.gpsimd.dma_start(
                        d_out[rep][:, offs[last_t] : offs[last_t] + e],
                        t_fout[:, offs[last_t] : offs[last_t] + e],
                    )

    return nc


def stage_inputs(inputs, bounds, maxL):
    """Build the 8 per-core input maps from the full problem inputs."""
    TM = sum(_tile_widths(maxL))
    f_pre_in = np.asarray(inputs["f_pre_in"], dtype=np.float32)
    b_pre_in = np.asarray(inputs["b_pre_in"], dtype=np.float32)
    bv_in = np.asarray(inputs["bv_in"], dtype=np.float32)
    Wf = np.asarray(inputs["Wf"], dtype=np.float32)
    bf = np.asarray(inputs["bf"], dtype=np.float32)
    Wb = np.asarray(inputs["Wb"], dtype=np.float32)
    Wbv = np.asarray(inputs["Wbv"], dtype=np.float32)
    bbv = np.asarray(inputs["bbv"], dtype=np.float32)
    Wo = np.asarray(inputs["Wo"], dtype=np.float32)
    bo = np.asarray(inputs["bo"], dtype=np.float32)

    # Folded output weights: out = A @ (bv@w_e)/de + Bm @ (bv@w_g)/dg + bo_tot
    A = Wo[:, :H] @ Wbv
    Bm = Wo[:, H:] @ Wbv
    bo_tot = bo + (Wo[:, :H] + Wo[:, H:]) @ bbv

    wpk = np.zeros((128, 256), np.float32)
    wpk[:, 0:64] = Wf.T
    wpk[:, 64:128] = Wb.T
    # final lhsT: rows 0-5 = A cols, 6 = bo_tot, 32-37 = Bm cols, 38 = 0
    wpk[0:6, 128:192] = A.T
    wpk[6, 128:192] = bo_tot
    wpk[0:6, 192:256] = Bm.T

    in_maps = []
    for c in range(N_CORES):
        s, e = int(bounds[c]), int(bounds[c + 1])
        L = e - s
        ft = np.zeros((FD, TM), np.float32)
        ft[:, :L] = f_pre_in[s:e].T
        bv8 = np.zeros((128, 4, 64), np.float32)
        for j in range(4):
            bv8[:, j, 0:BVD] = bv_in[c, :, 128 * j : 128 * (j + 1)].T
        bv8[:, :, 6] = 1.0       # -> psa row 6 = denominator (cat ones row)
        bv8[:, :, 32:39] = 1.0   # -> psa rows 32-38 = denominator block
        in_maps.append(
            {
                "wpk": wpk.astype(NP_BF16),
                "bfv": bf.reshape(H, 1).copy(),
                "ft": ft.astype(NP_BF16),
                "bp": b_pre_in[c].astype(NP_BF16),
                "bv8": bv8.astype(NP_FP8),
            }
        )
    return in_maps


def unstage_output(results, bounds, maxL):
    out = np.zeros((M, H), np.float32)
    for c in range(N_CORES):
        s, e = int(bounds[c]), int(bounds[c + 1])
        L = e - s
        outT = results[c]["outT"][0]
        out[s:e] = outT[:, :L].T.astype(np.float32)
    return out


_NC_CACHE = {}


def _program(maxL, reps=1):
    key = (maxL, reps)
    if key not in _NC_CACHE:
        _NC_CACHE[key] = build_program(maxL, reps=reps)
    return _NC_CACHE[key]


def kernel(**inputs):
    assert np.asarray(inputs["f_pre_in"]).shape == (M, FD)
    batch = np.asarray(inputs["f_pre_batch"]).astype(np.int64)
    bounds = np.searchsorted(batch, np.arange(B + 1))
    maxL = int(np.diff(bounds).max())
    in_maps = stage_inputs(inputs, bounds, maxL)
    nc = _program(maxL)
    res = run_bass_kernel_spmd(nc, in_maps, core_ids=list(range(N_CORES)))
    return unstage_output(res.results, bounds, maxL)


if __name__ == "__main__":
    rng = np.random.default_rng(0)
    demo = {
        "f_pre_in": rng.standard_normal((M, FD), dtype=np.float32),
        "f_pre_batch": np.sort(rng.integers(0, B, size=M)),
        "b_pre_in": rng.standard_normal((B, BD, N), dtype=np.float32),
        "bv_in": rng.standard_normal((B, BVD, N), dtype=np.float32),
        "Wf": rng.standard_normal((H, FD), dtype=np.float32) * 0.05,
        "bf": rng.standard_normal(H, dtype=np.float32) * 0.05,
        "Wb": rng.standard_normal((H, BD), dtype=np.float32) * 0.05,
        "bb": rng.standard_normal(H, dtype=np.float32) * 0.05,
        "Wbv": rng.standard_normal((H, BVD), dtype=np.float32) * 0.05,
        "bbv": rng.standard_normal(H, dtype=np.float32) * 0.05,
        "Wo": rng.standard_normal((H, 2 * H), dtype=np.float32) * 0.05,
        "bo": rng.standard_normal(H, dtype=np.float32) * 0.05,
    }
    out = kernel(**demo)
    print("kernel output", out.shape, out.dtype, float(np.abs(out).mean()))



# revision 35
# speedup vs baseline: 1.0082x; 1.0082x over previous
"""Trainium2 Bass kernel for the ragged-sequence cross-attention module.

Math (reference):
    f       = Wf @ f_pre_in.T + bf                      (H, M)
    b_feat  = Wb @ b_pre_in[g] + bb                     per graph (H, N)
    bv_feat = Wbv @ bv_in[g] + bbv                      per graph (H, N)
    w_euc   = softmax((b_feat[g,:32].T @ f[:32]) / 8)   per node over N
    w_geo   = softmax((b_feat[g,32:].T @ f[32:]) / 8)
    out     = [bv_feat[g] @ w_euc, bv_feat[g] @ w_geo] @ Wo.T + bo   (M, H)

Algebraic folds used here (host-side weight preprocessing only):
  * bb never enters: softmax over n is invariant to per-node constants, so
    s = (b^ + bb)^T (f^ + bf) == b^T (f^ + bf) up to a per-node constant.
  * Wbv/bbv/Wo/bo fold into A = Wo[:, :H] @ Wbv, Bm = Wo[:, H:] @ Wbv and
    bo_tot = bo + (Wo[:, :H] + Wo[:, H:]) @ bbv, since sum(softmax) == 1.
    The device then applies raw bv (6 channels) + a ones channel
    (denominator) and projects [e/de; 1; g/dg; 1] with a (14 -> 64) matmul.

Sharding: one core per graph (B == n_cores == 8).  Nodes are sorted by
graph, so core g takes graph g's contiguous node range, padded to
T = max_g ceil(len_g/128) m-tiles of 128 nodes.  Each core loads only its
own graph's b_pre/bv plus its node features: ~0.3 MB (bf16/fp8 staging).

Device pipeline per m-tile t (128 nodes, n = 512 boundary positions):
  scores  s[n, m] = b^T f^  -- 8 bf16 matmuls into PSUM (128, 2, 4, 128)
  exp     one ACT op (128, 1024): te = exp(s/8) -> fp8e4, DoubleRow layout
  apply   fp8 DoubleRow matmuls: lhsT = [bv(6) | ones x7 | 0 x3] (128,2,16)
          -> psum rows 0..5 = raw outs, rows 6..12 = denominator (replicated
          on 7 partitions so no partition-broadcast is ever needed)
  norm    DVE: reciprocal (7, ...) then two tensor_muls -> cat (14, t, 128)
          rows 0..5 = e/de, 6 = 1 (de/de), 7..12 = g/dg, 13 = 1
  fin     one (14 -> 64) bf16 matmul (bo_tot in row 6) -> PSUM -> DMA out

Norm/fin run in batches ([0:T-2], [T-2:T-1], [T-1:T]) so the serial tail
after the last exp is one small recip+mul+matmul+DMA chain.
"""

import sys

for _p in ("/opt/trn_rl_repo", "/root/.axon_site/_ro/trn_rl_repo"):
    if _p not in sys.path:
        sys.path.append(_p)

import numpy as np

import bass_rust

import concourse.bass as bass
import concourse.mybir as mybir
from concourse.bass_utils import run_bass_kernel_spmd
from concourse.tile import TileContext
from concourse.vector_clock import ScopedClock, VectorClock

F32 = mybir.dt.float32
BF16 = mybir.dt.bfloat16
FP8 = mybir.dt.float8e4

NP_BF16 = mybir.dt.np(BF16)
NP_FP8 = mybir.dt.np(FP8)

# Problem shapes (hardcoded per the harness contract).
M, B, N, FD, BD, BVD, H = 4096, 8, 512, 128, 128, 6, 64
H2 = H // 2
N_CORES = 8

# Apply matmuls run fp8 DoubleRow (2 k-tiles per pass).  Set False to fall
# back to plain fp8 single-row accumulation (4 matmuls per half).
USE_DR = True

# Tile indices whose exp runs on DVE as a Schraudolph affine map straight to
# fp8e4 bits (byte = round(s*log2e + 56.+c)), offloading the bottleneck ACT
# engine.  The mantissa-linear approximation costs ~3% extra weight noise on
# those tiles only.  Set to () to keep every exp on ACT.
EXP_DVE_TILES = ()
# fp8e4(v) bits = 8*(log2(v)+7) with mantissa-linear interpolation; the +0.36
# centers the 2^frac-1 vs frac error band.
SCHRAUDOLPH_MUL = 1.4426950408889634  # log2(e): byte per unit score (s/8*8)
SCHRAUDOLPH_ADD = 56.5 + 0.36  # +0.5 assumes truncating f32->u8 convert

# The walrus build in this environment rejects multiple semaphore waits
# on one instruction, so carry every wait on its own nop ahead of the real
# instruction.
_MAX_WAITS = 1


class _ChunkedDrainTileContext(TileContext):
    """The walrus build in this environment rejects >1 semaphore wait on a
    single instruction, which breaks TileContext's final drain (it carries
    one wait per touched proc).  Split those waits across one nop per proc;
    each engine executes serially, so a bare drain afterwards is
    equivalent."""

    _nop_uid = 0

    def _add_instruction(self, inst):
        si = inst.sync_info
        if (
            si is not None
            and si.on_wait
            and len(si.on_wait) > _MAX_WAITS
            and inst.engine != mybir.EngineType.Unassigned
        ):
            waits = list(si.on_wait)
            excess, keep = waits[:-_MAX_WAITS], waits[-_MAX_WAITS:]
            for i in range(0, len(excess), _MAX_WAITS):
                _ChunkedDrainTileContext._nop_uid += 1
                nop = mybir.InstNoOp(
                    name=f"splitw{_ChunkedDrainTileContext._nop_uid}", ins=[], outs=[]
                )
                nop.engine = inst.engine
                nop.sync_info = bass_rust.SyncInfo(
                    on_wait=excess[i : i + _MAX_WAITS], on_update=[]
                )
                super()._add_instruction(nop)
            inst.sync_info = bass_rust.SyncInfo(on_wait=keep, on_update=si.on_update)
        super()._add_instruction(inst)

    def _drain_and_barrier(self, tick_clock, wait_clock):
        nc = self.nc
        g = tick_clock.global_clock
        nprocs = len(g)
        for i in range(nprocs):
            if g[i] > 0:
                vc = VectorClock([0] * nprocs)
                vc.require_at_least(i, g[i])
                nop_inst = nc.sync.nop(nofuse=True, hint=f"drain_wait_p{i}")
                wait_clock.add_sem_waits(nop_inst.ins, ScopedClock({None: vc}))
        nc.sync.drain()
        nc.all_engine_barrier()
        assert self.sems is not None
        popped = nc._tile_sem_poison_stack.pop()
        assert popped is self._sem_poison
        nc.clear_and_free_semaphores(list(self.sems.allocated().values()))
        nc.all_engine_barrier()


def _tile_widths(maxL):
    """Column widths of the per-core m-tiles: all 128 wide.  Sub-128
    matmul widths abort on this hardware (PSUM write constraint the sim
    does not model), so the remainder tile stays 128 wide for every PE op
    and only its exp activation is sliced down (see build_program)."""
    return [128] * max(1, -(-maxL // 128))


def _batches(nt):
    """Group tile indices into norm/fin batches of <=2 tiles.  The last
    batch naturally ends up being the (small) remainder tile, keeping the
    post-last-exp serial tail short."""
    return [(i, min(i + 2, nt)) for i in range(0, nt, 2)]


def build_program(maxL, reps=1):
    """Build the per-core SPMD Bass program (identical on all 8 cores).

    maxL = max nodes per graph (sets the padded slot count).  reps>1
    repeats the pipeline writing to distinct output slices (timing only)."""
    nc = bass.Bass()
    widths = _tile_widths(maxL)
    offs = [0]
    for w in widths:
        offs.append(offs[-1] + w)
    TM = offs[-1]
    NT = len(widths)
    # The last tile is mostly padding (maxL mod 128 real nodes); its exp --
    # the expensive ACT op -- only covers the real columns, rounded up a
    # little.  Stale te bytes beyond that feed only discarded padding slots.
    wr = maxL - 128 * (NT - 1)
    exp_w = [128] * NT
    if 0 < wr < 128:
        exp_w[NT - 1] = min(128, -(-wr // 16) * 16)

    d_w = nc.declare_dram_parameter("wpk", [128, 256], BF16, isOutput=False)
    d_bf = nc.declare_dram_parameter("bfv", [H, 1], F32, isOutput=False)
    d_ft = nc.declare_dram_parameter("ft", [FD, TM], BF16, isOutput=False)
    d_bp = nc.declare_dram_parameter("bp", [BD, N], BF16, isOutput=False)
    d_bv = nc.declare_dram_parameter("bv8", [128, 4, 64], FP8, isOutput=False)
    d_out = nc.declare_dram_parameter("outT", [reps, H, TM], BF16, isOutput=True)

    DR = mybir.MatmulPerfMode.DoubleRow if USE_DR else None
    Exp = mybir.ActivationFunctionType.Exp

    with _ChunkedDrainTileContext(nc) as tc, nc.allow_low_precision(
        reason="bf16/fp8 rounding of fp32 data"
    ):
        with (
            tc.tile_pool(name="const", bufs=1) as cp,
            tc.tile_pool(name="work", bufs=2) as wkp,
            tc.tile_pool(name="te", bufs=3) as tep,
            tc.tile_pool(name="ps_s", bufs=2, space="PSUM") as pss,
            tc.tile_pool(name="ps_a", bufs=2, space="PSUM") as psap,
        ):
            # Preload the ACT Exp table at t=0 (a real first Exp would pay
            # the ~1.3us table load on the critical path).
            t_warm = cp.tile([1, 1], F32, tag="warm")
            nc.vector.memset(t_warm[:], 0.0)
            t_warm2 = cp.tile([1, 1], F32, tag="warm2")
            nc.scalar.activation(t_warm2[:], t_warm[:], Exp)

            # b_pre first on the SWDGE queue (bhat gates the first scores),
            # as two tiles so the half-0 matmul starts sooner.
            t_bpA = cp.tile([BD, N // 2], BF16, tag="bpA")
            nc.gpsimd.dma_start(t_bpA[:], d_bp[:, 0 : N // 2])
            t_bpB = cp.tile([BD, N // 2], BF16, tag="bpB")
            nc.gpsimd.dma_start(t_bpB[:], d_bp[:, N // 2 : N])
            t_bv = cp.tile([128, 4, 64], FP8, tag="bv8")
            nc.gpsimd.dma_start(t_bv[:], d_bv[:])
            t_bf = cp.tile([H, 1], F32, tag="bfv")
            nc.gpsimd.dma_start(t_bf[:], d_bf[:])
            t_w = cp.tile([128, 256], BF16, tag="wpk")
            nc.scalar.dma_start(t_w[:], d_w[:])

            t_wft = t_w[:, 0:64]
            t_wbt = t_w[:, 64:128]
            # final projection lhsT: rows 0-6 = [A.T; bo_tot], 32-38 = [Bm.T; 0]
            t_wfe = t_w[0:7, 128:192]
            t_wfg = t_w[0:7, 192:256]

            # fhat pieces: a small first piece so tile-0 scores start early.
            pieces = [(0, min(128, TM))]
            if TM > 128:
                pieces.append((128, TM))

            for rep in range(reps):
                # ---- per-rep node-feature load (SP queue, piecewise) ----
                t_ft = wkp.tile([FD, TM], BF16, tag="ft")
                for s, e in pieces:
                    nc.sync.dma_start(t_ft[:, s:e], d_ft[:, s:e])

                # ---- features ----
                # bhat first on PE (it gates the first scores); two separate
                # half-tiles so the first scores only wait on half 0's copy,
                # which itself overlaps the half-1 matmul
                t_bh0 = wkp.tile([H, N // 2], BF16, tag="bh0")
                t_bh1 = wkp.tile([H, N // 2], BF16, tag="bh1")
                ps_b0 = psap.tile([H, N // 2], F32, tag="finA", bufs=1)
                ps_b1 = psap.tile([H, N // 2], F32, tag="a")
                for ps_bh, t_bh, t_bph in (
                    (ps_b0, t_bh0, t_bpA),
                    (ps_b1, t_bh1, t_bpB),
                ):
                    nc.tensor.matmul(
                        ps_bh[:], t_wbt, t_bph[:], start=True, stop=True
                    )
                    # ACT is idle before the exps; keep DVE off this path
                    nc.scalar.copy(t_bh[:], ps_bh[:])

                def bhat(h0, j):  # noqa: B023 - per-rep closure
                    t_bh = t_bh0 if j < 2 else t_bh1
                    return t_bh[h0 : h0 + H2, 128 * (j % 2) : 128 * (j % 2 + 1)]

                # fhat in two tiles so early scores don't wait on the rest
                t_fh0 = wkp.tile([H, pieces[0][1]], BF16, tag="fh0")
                if TM > 128:
                    t_fh1 = wkp.tile([H, TM - 128], BF16, tag="fh1")
                else:
                    t_fh1 = None

                def fhat(h0, s, e):  # noqa: B023 - per-rep closure
                    if e <= 128:
                        return t_fh0[h0 : h0 + H2, s:e]
                    assert s >= 128
                    return t_fh1[h0 : h0 + H2, s - 128 : e - 128]

                for pi, ((s, e), t_fh) in enumerate(zip(pieces, (t_fh0, t_fh1))):
                    tg = "a" if pi == 0 else "finA"
                    ps_f = psap.tile([H, e - s], F32, tag=tg, bufs=2 - pi)
                    nc.tensor.matmul(
                        ps_f[:], t_wft, t_ft[:, s:e], start=True, stop=True
                    )
                    nc.vector.tensor_scalar_add(t_fh[:], ps_f[:], t_bf[:])

                # Natural order: every full exp (~1038ns) covers the next
                # tile's scores latency, so the ps_s ring of 2 never gaps,
                # and the (cheap, sliced) remainder tile lands LAST so the
                # post-last-exp serial tail is the short sliced chain.
                tile_order = list(range(NT))
                last_t = tile_order[-1]

                # Final projections land in two PSUM pieces: finA holds every
                # tile except the last-processed one (packed 128-col slots),
                # so its SBUF copies + out-DMAs can leave while the last
                # tile's norm still runs; finB (one tile) follows.  For
                # NT > 5 the split would overflow the 8 PSUM banks, so fall
                # back to one fin tile and a single tail copy/DMA.
                split_fin = 2 <= NT <= 5
                if split_fin:
                    nonlast = [t for t in tile_order if t != last_t]
                    ps_finB = psap.tile([H, 128], F32, tag="finB", bufs=1)
                else:
                    nonlast = list(tile_order)
                    ps_finB = None
                fcol = {t: 128 * i for i, t in enumerate(sorted(nonlast))}
                ps_finA = psap.tile(
                    [H, 128 * len(nonlast)], F32, tag="finA", bufs=1
                )

                # psa rows 0-5 = raw outs, 6 = denom, 32-38 = denom x7
                # (replicated via ones columns in bv8 so the reciprocal and
                # the tensor_muls each see quadrant-aligned partition bases)
                norm_tiles = {}

                def apply_norm_tile(t, te, hi=None):  # noqa: B023
                    # hi=None: both halves.  hi=0/1: just that half's apply
                    # matmuls + sliced recip/mul (the fins land at hi=1), so
                    # the long-tail tile's h0 norm overlaps its h1 exp.
                    w = widths[t]
                    if t not in norm_tiles:
                        norm_tiles[t] = (
                            psap.tile([64, 2, w], F32, tag="a", name=f"psa{t}"),
                            wkp.tile([64, 2, w], F32, tag="rcp", name=f"rcp{t}"),
                            wkp.tile([64, 2, w], BF16, tag="cat", name=f"cat{t}"),
                        )
                    t_psa, t_rcp, t_cat = norm_tiles[t]
                    his = range(2) if hi is None else [hi]
                    for h in his:
                        if USE_DR:
                            for jp in range(2):
                                nc.tensor.matmul(
                                    t_psa[0:64, h, :],
                                    t_bv[:, 2 * jp : 2 * jp + 2, :],
                                    te[:, h, 2 * jp : 2 * jp + 2, :],
                                    start=(jp == 0),
                                    stop=(jp == 1),
                                    perf_mode=DR,
                                )
                        else:
                            for j in range(4):
                                nc.tensor.matmul(
                                    t_psa[0:64, h, :],
                                    t_bv[:, j, :],
                                    te[:, h, j, :],
                                    start=(j == 0),
                                    stop=(j == 3),
                                )
                    # norm: cat (7, 2, w) rows 0-5 = x/dx, row 6 = 1, with the
                    # half index in the free dim.  The DVE ops only cover the
                    # real columns of the remainder tile; stale cat columns
                    # beyond feed discarded padding slots.
                    ew = exp_w[t]
                    hs = slice(None) if hi is None else slice(hi, hi + 1)
                    nc.vector.reciprocal(
                        t_rcp[32:39, hs, 0:ew], t_psa[32:39, hs, 0:ew]
                    )
                    if hi is None and ew < w:
                        nc.gpsimd.memset(t_cat[:, :, ew:w], 0.0)
                    nc.vector.tensor_mul(
                        t_cat[0:7, hs, 0:ew],
                        t_psa[0:7, hs, 0:ew],
                        t_rcp[32:39, hs, 0:ew],
                    )
                    if hi == 0:
                        return
                    if split_fin and t == last_t:
                        fin = ps_finB[:, 0:w]
                    else:
                        fin = ps_finA[:, fcol[t] : fcol[t] + w]
                    nc.tensor.matmul(
                        fin, t_wfe, t_cat[0:7, 0, :], start=True, stop=False
                    )
                    nc.tensor.matmul(
                        fin, t_wfg, t_cat[0:7, 1, :], start=False, stop=True
                    )

                # scores -> exp -> (apply+norm) with one-tile lookahead:
                # engines run their queues in order, so apply(t-1) is emitted
                # after scores(t) to keep PE busy during exp(t-1).
                # The long-tail tile (the last FULL one when the remainder
                # is last) runs its exp as two half ops so its h0 apply/norm
                # overlaps the h1 exp and the remainder tile's chain.
                tl = None  # half-split of the long-tail tile regressed:
                # ps_finA dependencies are whole-tile, so every out copy
                # waits for the last fin regardless; keep the simple path.
                pend = None  # (t, te) whose apply/norm is not yet emitted
                te_tl = None
                for t in tile_order:
                    w = widths[t]
                    ps_s = pss.tile([128, 2, 4, w], F32, tag="s")
                    for hi in range(2):
                        h0 = H2 * hi
                        for j in range(4):
                            nc.tensor.matmul(
                                ps_s[:, hi, j, :],
                                bhat(h0, j),
                                fhat(h0, offs[t], offs[t] + w),
                                start=True,
                                stop=True,
                            )
                    te = tep.tile([128, 2, 4, w], FP8, tag="te")
                    ew = exp_w[t]
                    if ew < w:
                        # padding columns the sliced exp skips: zero once on
                        # the idle Pool engine (their outputs are discarded,
                        # but stale bytes must not be NaN for the sim)
                        nc.gpsimd.memset(te[:, :, :, ew:w], 0.0)
                    if t == tl:
                        nc.scalar.activation(
                            te[:, 0:1, :, :], ps_s[:, 0:1, :, :], Exp, scale=0.125
                        )
                        if pend is not None:
                            apply_norm_tile(*pend)
                        nc.scalar.activation(
                            te[:, 1:2, :, :], ps_s[:, 1:2, :, :], Exp, scale=0.125
                        )
                        pend = None
                        te_tl = te
                        continue
                    if t in EXP_DVE_TILES:
                        nc.vector.tensor_scalar(
                            te[:, :, :, 0:ew].bitcast(mybir.dt.uint8),
                            ps_s[:, :, :, 0:ew],
                            SCHRAUDOLPH_MUL,
                            SCHRAUDOLPH_ADD,
                            mybir.AluOpType.mult,
                            mybir.AluOpType.add,
                        )
                    else:
                        nc.scalar.activation(
                            te[:, :, :, 0:ew], ps_s[:, :, :, 0:ew], Exp, scale=0.125
                        )
                    if pend is not None:
                        apply_norm_tile(*pend)
                    if te_tl is not None:
                        apply_norm_tile(tl, te_tl, hi=0)
                        apply_norm_tile(tl, te_tl, hi=1)
                        te_tl = None
                    pend = (t, te)
                if te_tl is not None:
                    apply_norm_tile(tl, te_tl, hi=0)
                    apply_norm_tile(tl, te_tl, hi=1)
                if pend is not None:
                    apply_norm_tile(*pend)

                # Piece-wise copies + SWDGE out-DMAs: finA's contiguous runs
                # leave as soon as their norms finish; finB (the last tile's
                # norm chain) goes last.
                t_fout = wkp.tile([H, TM], BF16, tag="fout")
                # finA leaves in two pieces so the big early piece's copy +
                # DMA never wait on the long-tail tile's norm chain.
                runs = []  # contiguous tile runs of finA
                for t in sorted(nonlast):
                    if runs and runs[-1][1] == t:
                        runs[-1][1] = t + 1
                    else:
                        runs.append([t, t + 1])
                pieces = []
                for t0, t1 in runs:
                    if t1 - t0 > 1 and t1 - 1 == tl:
                        pieces.extend([(t0, t1 - 1), (t1 - 1, t1)])
                    else:
                        pieces.append((t0, t1))
                for t0, t1 in pieces:
                    nc.scalar.copy(
                        t_fout[:, offs[t0] : offs[t1]],
                        ps_finA[:, fcol[t0] : fcol[t0] + offs[t1] - offs[t0]],
                    )
                    nc.sync.dma_start(
                        d_out[rep][:, offs[t0] : offs[t1]],
                        t_fout[:, offs[t0] : offs[t1]],
                    )
                if ps_finB is not None:
                    # DVE frees up exactly when the last norm finishes; the
                    # remainder's piece is sliced to its real columns.
                    e = exp_w[last_t]
                    nc.vector.tensor_copy(
                        t_fout[:, offs[last_t] : offs[last_t] + e],
                        ps_finB[:, 0:e],
                    )
                    nc.gpsimd.dma_start(
                        d_out[rep][:, offs[last_t] : offs[last_t] + e],
                        t_fout[:, offs[last_t] : offs[last_t] + e],
                    )

    return nc


def stage_inputs(inputs, bounds, maxL):
    """Build the 8 per-core input maps from the full problem inputs."""
    TM = sum(_tile_widths(maxL))
    f_pre_in = np.asarray(inputs["f_pre_in"], dtype=np.float32)
    b_pre_in = np.asarray(inputs["b_pre_in"], dtype=np.float32)
    bv_in = np.asarray(inputs["bv_in"], dtype=np.float32)
    Wf = np.asarray(inputs["Wf"], dtype=np.float32)
    bf = np.asarray(inputs["bf"], dtype=np.float32)
    Wb = np.asarray(inputs["Wb"], dtype=np.float32)
    Wbv = np.asarray(inputs["Wbv"], dtype=np.float32)
    bbv = np.asarray(inputs["bbv"], dtype=np.float32)
    Wo = np.asarray(inputs["Wo"], dtype=np.float32)
    bo = np.asarray(inputs["bo"], dtype=np.float32)

    # Folded output weights: out = A @ (bv@w_e)/de + Bm @ (bv@w_g)/dg + bo_tot
    A = Wo[:, :H] @ Wbv
    Bm = Wo[:, H:] @ Wbv
    bo_tot = bo + (Wo[:, :H] + Wo[:, H:]) @ bbv

    wpk = np.zeros((128, 256), np.float32)
    wpk[:, 0:64] = Wf.T
    wpk[:, 64:128] = Wb.T
    # final lhsT: rows 0-5 = A cols, 6 = bo_tot, 32-37 = Bm cols, 38 = 0
    wpk[0:6, 128:192] = A.T
    wpk[6, 128:192] = bo_tot
    wpk[0:6, 192:256] = Bm.T

    in_maps = []
    for c in range(N_CORES):
        s, e = int(bounds[c]), int(bounds[c + 1])
        L = e - s
        ft = np.zeros((FD, TM), np.float32)
        ft[:, :L] = f_pre_in[s:e].T
        bv8 = np.zeros((128, 4, 64), np.float32)
        for j in range(4):
            bv8[:, j, 0:BVD] = bv_in[c, :, 128 * j : 128 * (j + 1)].T
        bv8[:, :, 6] = 1.0       # -> psa row 6 = denominator (cat ones row)
        bv8[:, :, 32:39] = 1.0   # -> psa rows 32-38 = denominator block
        in_maps.append(
            {
                "wpk": wpk.astype(NP_BF16),
                "bfv": bf.reshape(H, 1).copy(),
                "ft": ft.astype(NP_BF16),
                "bp": b_pre_in[c].astype(NP_BF16),
                "bv8": bv8.astype(NP_FP8),
            }
        )
    return in_maps


def unstage_output(results, bounds, maxL):
    out = np.zeros((M, H), np.float32)
    for c in range(N_CORES):
        s, e = int(bounds[c]), int(bounds[c + 1])
        L = e - s
        outT = results[c]["outT"][0]
        out[s:e] = outT[:, :L].T.astype(np.float32)
    return out


_NC_CACHE = {}


def _program(maxL, reps=1):
    key = (maxL, reps)
    if key not in _NC_CACHE:
        _NC_CACHE[key] = build_program(maxL, reps=reps)
    return _NC_CACHE[key]


def kernel(**inputs):
    assert np.asarray(inputs["f_pre_in"]).shape == (M, FD)
    batch = np.asarray(inputs["f_pre_batch"]).astype(np.int64)
    bounds = np.searchsorted(batch, np.arange(B + 1))
    maxL = int(np.diff(bounds).max())
    in_maps = stage_inputs(inputs, bounds, maxL)
    nc = _program(maxL)
    res = run_bass_kernel_spmd(nc, in_maps, core_ids=list(range(N_CORES)))
    return unstage_output(res.results, bounds, maxL)


if __name__ == "__main__":
    rng = np.random.default_rng(0)
    demo = {
        "f_pre_in": rng.standard_normal((M, FD), dtype=np.float32),
        "f_pre_batch": np.sort(rng.integers(0, B, size=M)),
        "b_pre_in": rng.standard_normal((B, BD, N), dtype=np.float32),
        "bv_in": rng.standard_normal((B, BVD, N), dtype=np.float32),
        "Wf": rng.standard_normal((H, FD), dtype=np.float32) * 0.05,
        "bf": rng.standard_normal(H, dtype=np.float32) * 0.05,
        "Wb": rng.standard_normal((H, BD), dtype=np.float32) * 0.05,
        "bb": rng.standard_normal(H, dtype=np.float32) * 0.05,
        "Wbv": rng.standard_normal((H, BVD), dtype=np.float32) * 0.05,
        "bbv": rng.standard_normal(H, dtype=np.float32) * 0.05,
        "Wo": rng.standard_normal((H, 2 * H), dtype=np.float32) * 0.05,
        "bo": rng.standard_normal(H, dtype=np.float32) * 0.05,
    }
    out = kernel(**demo)
    print("kernel output", out.shape, out.dtype, float(np.abs(out).mean()))

